# revision 9
# baseline (speedup 1.0000x reference)
"""AxialAttention Trainium2 Bass kernel (batched-LDWEIGHTS rewrite).

Problem: q,k,v of shape (4, 8, 16, 32, 32, 64) = (b, heads, t, h, w, d),
attention along the h axis (axis 3), softmax over keys, out same shape.

512 independent slabs (b, heads, t), each a batch of w=32 length-32
attention problems with head dim 64.  64 slabs per core, processed in
quads (4 slabs).  Within a quad, slab s = 2*sl + sh: sl picks the
64-partition row half of QT/KT, sh picks the free-dim half.

Key ideas vs the naive version:
  - Host pre-lays-out Q/K transposed ([quad, (sl,d), (sh,w,h)]) and V
    augmented with a ones column ([quad, (om,h), (s,half,g,66)]), so every
    DMA is a single fully-contiguous 128-partition transfer and no
    on-chip transposes are needed.
  - Scores use K=64 contraction; ONE standalone 128x128 LDWEIGHTS feeds
    8 non-self-loading matmuls (2 row-pairs x 4 col groups).  PV: ONE
    strided-col LDWEIGHTS feeds 16 matmuls at 16 tile positions.
    Constraint: concurrent matmuls from different row groups must write
    different PSUM banks (bank per row group).
  - exp on full 128-partition [128,256] tiles; softmax denominator via
    the fused ones column; normalization as a DVE tensor_mul reading
    PSUM directly; output stored as bf16 and upcast on the host.
  - All tensor-engine instructions are chained with nosync deps so the
    tile scheduler preserves the LDW->matmul pairing.
"""

import os
import sys
import numpy as np

for _p in ("/root/.axon_site/_ro/trn_rl_repo", "/opt/trn_rl_repo"):
    if os.path.isdir(_p) and _p not in sys.path:
        sys.path.append(_p)

B, NH, T, H, W, D = 4, 8, 16, 32, 32, 64
N_CORES = 8
NSLAB = B * NH * T          # 512
NSLAB_CORE = NSLAB // N_CORES  # 64
NQ = NSLAB_CORE // 4        # 16 quads per core
NQ_ALL = NSLAB // 4         # 128 quads globally
VST = 66                    # 64 d + 1 one + 1 pad

_CACHED_NC = None


def _build_nc():
    import concourse.bacc as bacc
    import concourse.mybir as mybir
    import concourse.tile as tile_mod
    from concourse import tile
    from concourse.tile_rust import add_dep_helper

    dt = mybir.dt

    # The tile legalizer pairs EVERY matmul with its own InstLdweights.
    # Our matmuls use weights already loaded by a shared standalone
    # ldweights (one 128x128 load feeding 8-16 tile-position matmuls), so
    # those per-matmul loads are pure overhead (~27ns each, ~110us/core).
    # Strip them right after legalize, folding their deps into the matmul.
    noldw_names = set()
    orig_legalize = tile_mod.tile_legalize

    def legalize_strip(obib, nc_):
        out = orig_legalize(obib, nc_)
        for bb in list(out.keys()):
            insts = out[bb]
            keep = []
            i, n = 0, len(insts)
            while i < n:
                inst = insts[i]
                if (isinstance(inst, mybir.InstLdweights) and i + 1 < n
                        and isinstance(insts[i + 1], mybir.InstMatmult)
                        and insts[i + 1].name in noldw_names):
                    nxt = insts[i + 1]
                    nxt.merge_dependencies_from(inst)
                    keep.append(nxt)
                    i += 2
                    continue
                keep.append(inst)
                i += 1
            out[bb] = keep
        return out

    tile_mod.tile_legalize = legalize_strip

    nc = bacc.Bacc("TRN2", target_bir_lowering=False, debug=False,
                   num_devices=N_CORES)
    qt_in = nc.dram_tensor("qt_in", [NQ, 128, 2048], dt.bfloat16,
                           kind="ExternalInput").ap()
    kt_in = nc.dram_tensor("kt_in", [NQ, 128, 2048], dt.bfloat16,
                           kind="ExternalInput").ap()
    v_in = nc.dram_tensor("v_in", [NQ, 128, 4 * 2 * 4 * VST], dt.bfloat16,
                          kind="ExternalInput").ap()
    o_out = nc.dram_tensor("o_out", [NQ, 128, 2048], dt.bfloat16,
                           kind="ExternalOutput").ap()

    scale = 1.0 / float(np.sqrt(D))

    with tile.TileContext(nc) as tc:
        with tc.tile_pool(name="io", bufs=2) as io_pool, \
             tc.tile_pool(name="ee", bufs=2) as e_pool, \
             tc.tile_pool(name="rr", bufs=2) as r_pool, \
             tc.tile_pool(name="oo", bufs=2) as o_pool, \
             tc.tile_pool(name="ps_sc", bufs=2, space="PSUM") as ps_sc, \
             tc.tile_pool(name="ps_pv", bufs=1, space="PSUM") as ps_pv:

            chain = [None]

            def tchain(bi):
                inst = bi.ins if hasattr(bi, "ins") else bi
                if chain[0] is not None:
                    add_dep_helper(inst, chain[0], sync=False,
                                   reason="pe order")
                chain[0] = inst
                return bi

            gstate = {}

            def emit_group_loads(gr, split_first=False):
                # one load group = 4 quads; big DMAs amortize the ~2us
                # per-transfer completion latency on the DGE rings.
                QT4 = io_pool.tile([128, 4, 2, W, H], dt.bfloat16, name="QT4")
                KT4 = io_pool.tile([128, 4, 2, W, H], dt.bfloat16, name="KT4")
                V4 = io_pool.tile([128, 4, 4, 2, 4, VST], dt.bfloat16,
                                  name="V4")
                q0 = 4 * gr
                if split_first:
                    # group 0: per-quad DMAs so compute starts sooner
                    for j in range(4):
                        nc.sync.dma_start(
                            out=KT4[:, j],
                            in_=kt_in[q0 + j].rearrange(
                                "p (a w h) -> p a w h", a=2, w=W))
                        nc.sync.dma_start(
                            out=QT4[:, j],
                            in_=qt_in[q0 + j].rearrange(
                                "p (a w h) -> p a w h", a=2, w=W))
                        nc.scalar.dma_start(
                            out=V4[:, j],
                            in_=v_in[q0 + j].rearrange(
                                "p (s f g x) -> p s f g x", s=4, f=2, g=4))
                else:
                    nc.sync.dma_start(
                        out=QT4[:, :, :, :, :],
                        in_=qt_in[q0:q0 + 4].rearrange(
                            "n p (a w h) -> p n a w h", a=2, w=W))
                    nc.sync.dma_start(
                        out=KT4[:, :, :, :, :],
                        in_=kt_in[q0:q0 + 4].rearrange(
                            "n p (a w h) -> p n a w h", a=2, w=W))
                    nc.scalar.dma_start(
                        out=V4[:, :, :, :, :, :],
                        in_=v_in[q0:q0 + 4].rearrange(
                            "n p (s f g x) -> p n s f g x", s=4, f=2, g=4))
                OUT4 = o_pool.tile([128, 4, 2, 4, 4, D], dt.bfloat16,
                                   name="OUT4")
                R4 = r_pool.tile([128, 4, 2, 4, 4], dt.float32, name="R4")
                gstate[gr] = dict(QT4=QT4, KT4=KT4, V4=V4, OUT4=OUT4, R4=R4)

            def qview(qi):
                g = gstate[qi // 4]
                j = qi % 4
                return dict(QT=g["QT4"][:, j], KT=g["KT4"][:, j],
                            V=g["V4"][:, j], OUT=g["OUT4"][:, j],
                            R=g["R4"][:, j])

            def emit_scores(qi, half):
                st = qview(qi)
                QT, KT = st["QT"], st["KT"]
                psc = [ps_sc.tile([128, 2, 4, H], dt.float32,
                                  name=f"psc{sl}") for sl in range(2)]
                E = e_pool.tile([128, 4, 4, H], dt.bfloat16, name="E")
                w0 = 16 * half
                for sh in range(2):
                    for g in range(4):
                        wb = w0 + 4 * g
                        tchain(nc.tensor.ldweights(KT[:, sh, wb:wb + 4, :]))
                        for sl in range(2):
                            for om in range(4):
                                w = wb + om
                                mm = nc.tensor.matmul(
                                    psc[sl][32 * om:32 * om + 32, sh, g, :],
                                    lhsT=KT[64 * sl:64 * sl + 64, sh, w, :],
                                    rhs=QT[64 * sl:64 * sl + 64, sh, w, :],
                                    start=True, stop=True,
                                    tile_position=(64 * sl, 32 * om))
                                noldw_names.add(mm.ins.name)
                                tchain(mm)
                # E layout is (g, s, q) so each per-g PV weight load reads a
                # CONTIGUOUS 128-col block (strided-col ldweights mis-lowers).
                for sl in range(2):
                    nc.scalar.activation(
                        E[:, :, 2 * sl:2 * sl + 2, :],
                        psc[sl][:, :, :, :].rearrange("p s g q -> p g s q"),
                        mybir.ActivationFunctionType.Exp, scale=scale)
                return E

            def emit_pv(qi, half, E):
                st = qview(qi)
                V, OUT, R = st["V"], st["OUT"], st["R"]
                psv = [ps_pv.tile([128, 4, D + 1], dt.float32,
                                  name=f"psv{om}") for om in range(4)]
                for g in range(4):
                    tchain(nc.tensor.ldweights(E[:, g, :, :]))
                    for om in range(4):
                        for s in range(4):
                            mm = nc.tensor.matmul(
                                psv[om][32 * s:32 * s + 32, g, 0:D + 1],
                                lhsT=E[32 * om:32 * om + 32, g, s, :],
                                rhs=V[32 * om:32 * om + 32, s, half, g,
                                      0:D + 1],
                                start=True, stop=True,
                                tile_position=(32 * om, 32 * s))
                            noldw_names.add(mm.ins.name)
                            tchain(mm)
                for om in range(4):
                    nc.vector.reciprocal(R[:, half, :, om],
                                         psv[om][:, :, D])
                    nc.vector.tensor_mul(
                        OUT[:, half, :, om, :],
                        psv[om][:, :, 0:D],
                        R[:, half, :, om, None].broadcast_to([128, 4, D]))

            def emit_store_half(qi, half):
                st = qview(qi)
                nc.scalar.dma_start(
                    out=o_out[qi, :, 1024 * half:1024 * half + 1024],
                    in_=st["OUT"][:, half, :, :, :])

            emit_group_loads(0, split_first=True)
            pending = None
            for t in range(2 * NQ):
                qi, half = divmod(t, 2)
                if half == 0 and qi % 4 == 0 and qi // 4 + 1 < NQ // 4:
                    emit_group_loads(qi // 4 + 1)
                E = emit_scores(qi, half)
                if pending is not None:
                    pqi, phalf, pE = pending
                    emit_pv(pqi, phalf, pE)
                    emit_store_half(pqi, phalf)
                pending = (qi, half, E)
            pqi, phalf, pE = pending
            emit_pv(pqi, phalf, pE)
            emit_store_half(pqi, phalf)
    tile_mod.tile_legalize = orig_legalize
    nc.compile()
    return nc


def _get_nc():
    global _CACHED_NC
    if _CACHED_NC is None:
        _CACHED_NC = _build_nc()
    return _CACHED_NC


def kernel(q, k, v, decode_step=0, decode_idx=0, _trace=False):
    from concourse.bass_utils import run_bass_kernel_spmd
    import ml_dtypes

    bf16 = ml_dtypes.bfloat16

    q = np.asarray(q, dtype=np.float32).reshape(NSLAB, H, W, D)
    k = np.asarray(k, dtype=np.float32).reshape(NSLAB, H, W, D)
    v = np.asarray(v, dtype=np.float32).reshape(NSLAB, H, W, D)

    # QT/KT: [quad, (sl, d)=128, (sh, w, h)=2048]
    qg = q.reshape(NQ_ALL, 2, 2, H, W, D).astype(bf16)
    qt = np.ascontiguousarray(qg.transpose(0, 1, 5, 2, 4, 3)) \
        .reshape(NQ_ALL, 128, 2048)
    kg = k.reshape(NQ_ALL, 2, 2, H, W, D).astype(bf16)
    kt = np.ascontiguousarray(kg.transpose(0, 1, 5, 2, 4, 3)) \
        .reshape(NQ_ALL, 128, 2048)
    # V: [quad, (om, h)=128, (s, half, g, VST)]; x=64 is the ones column
    vg = v.reshape(NQ_ALL, 4, H, 2, 4, 4, D).astype(bf16)
    v_aug = np.empty((NQ_ALL, 4, H, 4, 2, 4, VST), dtype=bf16)
    v_aug[..., :D] = vg.transpose(0, 5, 2, 1, 3, 4, 6)
    v_aug[..., D] = 1.0
    v_aug[..., D + 1:] = 0.0
    v_pre = v_aug.reshape(NQ_ALL, 128, 4 * 2 * 4 * VST)

    nc = _get_nc()
    in_maps = []
    for c in range(N_CORES):
        sl = slice(c * NQ, (c + 1) * NQ)
        in_maps.append({
            "qt_in": qt[sl],
            "kt_in": kt[sl],
            "v_in": v_pre[sl],
        })
    res = run_bass_kernel_spmd(nc, in_maps, core_ids=list(range(N_CORES)),
                               trace=_trace)
    o = np.concatenate([r["o_out"] for r in res.results], axis=0)
    # o: [quad, (s, q)=128, (half, g, om, d)=2048] == [slab, h, w, d]
    out = o.reshape(NSLAB, H, W, D).astype(np.float32)
    out = out.reshape(B, NH, T, H, W, D)
    if _trace:
        return out, res
    return out


if __name__ == "__main__":
    rng = np.random.default_rng(0)
    shape = (B, NH, T, H, W, D)
    q = rng.standard_normal(shape, dtype=np.float32)
    k = rng.standard_normal(shape, dtype=np.float32)
    v = rng.standard_normal(shape, dtype=np.float32)
    out = kernel(q, k, v)
    print("kernel ran, out shape", out.shape)
